# revision 36
# baseline (speedup 1.0000x reference)
"""Causal multi-head attention (nn_Attention_87840671138123) on 8 trn2 NeuronCores.

Problem (B=2, S=2048, D=1024, H=16 heads, E=64 head_dim), fp32:
    Q = einsum('bsd,hde->bhse', q, W_q)   (same for K, V)
    scores = Q @ K^T / sqrt(D), causal mask, softmax
    attn = probs @ V  -> [B, S, D] (head-major concat)
    out = attn @ W_o.T

Sharding: core = 4*b + quad. Each core handles batch b and a quad of 4 heads,
computes out_part = attn_quad @ W_o.T[quad rows, :], and the host sums the 4
partials per batch (the output-projection all-reduce done at gather time).

v2 schedule (vs the 233us baseline): the attention inner loop is
ACT(exp)-throughput-bound, so all projection matmuls for s-tile j+1 and the
output-projection matmuls for earlier tiles are emitted as *fillers*
interleaved into tile j's score/attn matmul stream; the attnV pair for block
cb is emitted one block behind its scores (skew) so exp latency is hidden.
Epilogue: reciprocal_approx_fast on the PSUM denominator row (lane-locked),
one gpsimd partition_broadcast, one multiply straight into attnG - no
copy+DMA+6-pass reciprocal chain.  V_aug for head pair h2=1 keeps its values
in PSUM partitions 64..127 (ones col at 63) so both normalize multiplies are
lane-aligned with attnG.  Output is written fp16 (cast is half the vector
time of an f32 copy, and half the HBM traffic); the host accumulates in f32.

Numerics: Q/K path bf16, V path fp16 (see baseline notes: fp32r trips the
chip power throttle).  exp folds the 1/sqrt(1024) scale.  fp16 out adds
~2e-4 relative error; harness gate is 2e-2.
"""

import ml_dtypes
import numpy as np

import concourse.bass as bass
import concourse.tile as tile
from concourse import bacc, mybir
from concourse.bass_utils import run_bass_kernel_spmd

B, S, D, H, E = 2, 2048, 1024, 16, 64
P = 128
NCORES = 8
SJ = 512            # s-tile width
NJ = S // SJ        # 4 s-tiles
ND = D // P         # 8 d-chunks
NT = S // P         # 16 t-chunks
f32 = mybir.dt.float32
bf16 = mybir.dt.bfloat16
fp16 = mybir.dt.float16
fp8 = mybir.dt.float8e4
EXP = mybir.ActivationFunctionType.Exp
MULT = mybir.AluOpType.mult
DROW = mybir.MatmulPerfMode.DoubleRow

X_DT = fp8          # q/k inputs + Wq/Wk: fp8 DoubleRow projections (2x PE).
W_SCALE = 8.0       # Wq/Wk pre-scaled by 8 to clear the fp8 subnormal floor;
                    # folded back via the exp scale (scores carry 64x).
QK_DT = bf16        # QT/KT activations, scores matmul
V_DT = fp16         # v input, Wv, V_aug, expT, attnG, WoT
OUT_DT = fp16       # device output; host accumulates in f32

_NP_OF = {bf16: ml_dtypes.bfloat16, fp16: np.float16, f32: np.float32,
          fp8: ml_dtypes.float8_e4m3}

_NC_CACHE = []


def _patch_ldw_opt():
    """Enable walrus LDWEIGHTS optimization (fast weight load).

    bass_utils.bir_verify_and_optimise hardcodes --enable-ldw-opt=false;
    wrap it to rewrite the flag. Verified numerically by the test harness.
    """
    from concourse import bass_utils as _bu
    if getattr(_bu, "_ldw_patched", False):
        return
    _orig_run = _bu.run_command

    def _run(argv, **kw):
        argv = ["--enable-ldw-opt=true" if a == "--enable-ldw-opt=false" else a
                for a in argv]
        return _orig_run(argv, **kw)

    _bu.run_command = _run
    _bu._ldw_patched = True


def _build():
    nc = bacc.Bacc("TRN2", target_bir_lowering=False, debug=False)

    # all inputs are host-pre-tiled to the exact SBUF layout so every DMA
    # moves long contiguous rows (512B rows run at ~19GB/s vs ~350GB/s for
    # 4-8KB rows)
    qT_d = nc.dram_tensor("qT", [NJ, P, ND, SJ], X_DT, kind="ExternalInput")
    kT_d = nc.dram_tensor("kT", [NJ, P, ND, SJ], X_DT, kind="ExternalInput")
    vT_d = nc.dram_tensor("vT", [NJ, P, ND, SJ], V_DT, kind="ExternalInput")
    wq_d = nc.dram_tensor("wq", [P, ND * 4 * E], X_DT, kind="ExternalInput")
    wk_d = nc.dram_tensor("wk", [P, ND * 4 * E], X_DT, kind="ExternalInput")
    wv_d = nc.dram_tensor("wv", [P, ND * 4 * E], V_DT, kind="ExternalInput")
    wot_d = nc.dram_tensor("wot", [P, 2 * D], V_DT, kind="ExternalInput")
    tri_d = nc.dram_tensor("tri", [P, P], V_DT, kind="ExternalInput")
    out_d = nc.dram_tensor("out", [S, D], OUT_DT, kind="ExternalOutput")

    with tile.TileContext(nc) as tc:
        with (
            tc.tile_pool(name="pers", bufs=1) as pers,
            tc.tile_pool(name="xt", bufs=2) as xt_pool,
            tc.tile_pool(name="ex", bufs=6) as ex_pool,
            tc.tile_pool(name="sm", bufs=4) as sm_pool,
            tc.tile_pool(name="ot", bufs=3) as ot_pool,
            tc.tile_pool(name="pj", bufs=2, space="PSUM") as pj_pool,
            tc.tile_pool(name="sc", bufs=2, space="PSUM") as sc_pool,
            tc.tile_pool(name="at", bufs=2, space="PSUM") as at_pool,
        ):
            # ---- persistent weights / constants ----
            wq_sb = pers.tile([P, ND, 4 * E], X_DT, name="wq_sb")
            wk_sb = pers.tile([P, ND, 4 * E], X_DT, name="wk_sb")
            wv_sb = pers.tile([P, ND, 4 * E], V_DT, name="wv_sb")
            wot_sb = pers.tile([P, 2, D], V_DT, name="wot_sb")
            tri_sb = pers.tile([P, P], V_DT, name="tri_sb")

            # ---- persistent activations ----
            QT = [pers.tile([P, S], QK_DT, name=f"QT{g}") for g in range(2)]
            # Per-head KT zero-padded to 128 partitions (rows of the other
            # head hold ~1e-20) so the scores matmul runs K=128: full PE rows
            # keep the HAM activity monitor from dropping the clock to 1.2GHz.
            KTH = [[pers.tile([P, S], QK_DT, name=f"KT{g}{h2}") for h2 in range(2)]
                   for g in range(2)]
            # V_aug blocks indexed [t, h2, g]: V in cols 0..63, ones at col 64
            # (denominator -> PSUM partition 64).  All pad cols hold 1.0 (PE
            # activity + they only feed unused PSUM partitions).  NB hardware
            # gpsimd/custom-DVE ucode does not honor partition offsets on
            # some APs, so the epilogue only uses baseline-proven shapes.
            Vall = pers.tile([P, NT, 2, 2, P], V_DT, name="Vall")
            attnG = [pers.tile([P, S], V_DT, name=f"attnG{g}") for g in range(2)]

            # weight DMAs ride the Activation HWDGE queue so they overlap the
            # x-input DMAs on the SP/Sync queue during startup
            nc.scalar.dma_start(wq_sb[:], wq_d.ap())
            nc.scalar.dma_start(wk_sb[:], wk_d.ap())
            nc.scalar.dma_start(wv_sb[:], wv_d.ap())
            nc.scalar.dma_start(tri_sb[:], tri_d.ap())
            nc.scalar.dma_start(wot_sb[:], wot_d.ap())

            # ---------------- helpers ----------------
            def start_x_dma(j):
                xq = xt_pool.tile([P, ND, SJ], X_DT, tag="xq", name=f"xq{j}")
                nc.sync.dma_start(xq[:], qT_d.ap()[j])
                xk = xt_pool.tile([P, ND, SJ], X_DT, tag="xk", name=f"xk{j}")
                nc.sync.dma_start(xk[:], kT_d.ap()[j])
                xv = xt_pool.tile([P, ND, SJ], V_DT, tag="xv", name=f"xv{j}")
                nc.sync.dma_start(xv[:], vT_d.ap()[j])
                return xq, xk, xv

            # Q/K projections run fp8 DoubleRow: each matmul contracts two
            # d-chunks (K=256) at 2 elem/partition/cycle -> half the PE time
            def q_step(c, g, j, js, xq, cell):
                if c == 0:
                    cell['t'] = pj_pool.tile([P, SJ], f32, tag="pj",
                                             name=f"pq{j}{g}")
                nc.tensor.matmul(cell['t'][:],
                                 wq_sb[:, 2 * c:2 * c + 2, bass.ts(g, P)],
                                 xq[:, 2 * c:2 * c + 2, :],
                                 start=(c == 0), stop=(c == ND // 2 - 1),
                                 perf_mode=DROW)
                if c == ND // 2 - 1:
                    nc.vector.tensor_copy(QT[g][:, js], cell['t'][:])

            def k_step(c, g, j, js, xk, cell):
                if c == 0:
                    cell['t'] = pj_pool.tile([P, SJ], f32, tag="pj",
                                             name=f"pk{j}{g}")
                nc.tensor.matmul(cell['t'][:],
                                 wk_sb[:, 2 * c:2 * c + 2, bass.ts(g, P)],
                                 xk[:, 2 * c:2 * c + 2, :],
                                 start=(c == 0), stop=(c == ND // 2 - 1),
                                 perf_mode=DROW)
                if c == ND // 2 - 1:
                    nc.vector.tensor_copy(KTH[g][0][0:E, js], cell['t'][0:E, :])
                    nc.vector.tensor_copy(KTH[g][1][E:2 * E, js],
                                          cell['t'][E:2 * E, :])

            def v_step(c, u, j, xv, cell):
                if c == 0:
                    cell['t'] = pj_pool.tile([P, 2, 2, E], f32, tag="pj",
                                             name=f"pv{j}{u}")
                nc.tensor.matmul(cell['t'][:], xv[:, c, bass.ts(u, P)],
                                 wv_sb[:, c, :], start=(c == 0), stop=(c == ND - 1))
                if c == ND - 1:
                    t = 4 * j + u
                    pv = cell['t']
                    # one cast for all four (h2, g) blocks: V into cols 0..63
                    nc.vector.tensor_copy(
                        Vall[:, t, :, :, 0:E],
                        pv[:].rearrange("p g h e -> p h g e"))

            def proj_fillers(j, init=False):
                x = start_x_dma(j)
                xq, xk, xv = x
                js = slice(j * SJ, (j + 1) * SJ)
                fl = []
                for g in range(2):
                    cell = {}
                    for c in range(ND // 2):
                        fl.append(lambda c=c, g=g, cell=cell:
                                  q_step(c, g, j, js, xq, cell))
                if init:
                    # one-time pads, placed here so they don't delay the
                    # first QT casts on the vector queue
                    for g in range(2):
                        fl.append(lambda g=g: nc.vector.memset(
                            KTH[g][0][E:2 * E, :], 1e-20))
                        fl.append(lambda g=g: nc.vector.memset(
                            KTH[g][1][0:E, :], 1e-20))
                for g in range(2):
                    cell = {}
                    for c in range(ND // 2):
                        fl.append(lambda c=c, g=g, cell=cell:
                                  k_step(c, g, j, js, xk, cell))
                if init:
                    # V_aug pad cols (ones col E + 1.0 filler above it); the
                    # V value cols are fully overwritten by the casts
                    fl.append(lambda: nc.vector.memset(
                        Vall[:, :, :, :, E:P], 1.0))
                for u in range(SJ // P):
                    cell = {}
                    for c in range(ND):
                        fl.append(lambda c=c, u=u, cell=cell:
                                  v_step(c, u, j, xv, cell))
                return fl

            def o_step(si, no, k, cell):
                if k == 0:
                    cell['t'] = pj_pool.tile([P, SJ], f32, tag="pj",
                                             name=f"po{si}{no}")
                    if no == 0:
                        cell['ot'] = ot_pool.tile([P, D], OUT_DT, tag="ot",
                                                  name=f"ot{si}")
                nc.tensor.matmul(cell['t'][:], attnG[k][:, bass.ts(si, P)],
                                 wot_sb[:, k, bass.ts(no, SJ)],
                                 start=(k == 0), stop=(k == 1))
                if k == 1:
                    nc.vector.tensor_copy(cell['ot'][:, bass.ts(no, SJ)],
                                          cell['t'][:])
                    if no == 1:
                        # one 2KB-row DMA per 128-row output stripe
                        nc.sync.dma_start(out_d.ap()[bass.ts(si, P), :],
                                          cell['ot'][:])

            def outproj_fillers(jp):
                fl = []
                for u in range(SJ // P):
                    si = 4 * jp + u
                    cell = {}  # shared per si: 'ot' spans both no-chunks
                    for no in range(2):
                        for k in range(2):
                            fl.append(lambda si=si, no=no, k=k, cell=cell:
                                      o_step(si, no, k, cell))
                return fl

            def epilogue(j, g, atp):
                js = slice(j * SJ, (j + 1) * SJ)
                for h2 in (1, 0):
                    den = sm_pool.tile([E + 1, SJ], f32, tag="den",
                                       name=f"den{j}{g}{h2}")
                    rec0 = sm_pool.tile([1, SJ], f32, tag="rec0",
                                        name=f"rec0{j}{g}{h2}")
                    # fast-free: pull raw attn + den out of PSUM immediately
                    # (two cheap copies) so the at-bank recycles in <1us and
                    # the reciprocal chain runs from SBUF staging
                    stg = sm_pool.tile([E, SJ], V_DT, tag="stg",
                                       name=f"stg{j}{g}{h2}")
                    nc.vector.tensor_copy(stg[:], atp[h2][0:E, :])
                    nc.vector.tensor_copy(den[E:E + 1, :], atp[h2][E:E + 1, :])
                    nc.sync.dma_start(rec0[:], den[E:E + 1, :])
                    # 1/den: 18-bit accurate, ~5x faster than the 6-pass
                    # reciprocal (den >= 1, far from the undefined edge cases)
                    nc.vector.reciprocal_approx_fast(rec0[:], rec0[:])
                    recb = sm_pool.tile([E, SJ], f32, tag="recb",
                                        name=f"recb{j}{g}{h2}")
                    nc.gpsimd.partition_broadcast(recb[:], rec0[:])
                    if h2 == 0:
                        nc.vector.tensor_tensor(
                            attnG[g][0:E, js], stg[:], recb[:], MULT)
                    else:
                        ah = sm_pool.tile([E, SJ], V_DT, tag="ah",
                                          name=f"ah{j}{g}")
                        nc.vector.tensor_tensor(ah[:], stg[:], recb[:], MULT)
                        nc.sync.dma_start(attnG[g][E:2 * E, js], ah[:])

            def attention(j, fillers):
                nblk = 4 * j + 4
                slots = 2 * nblk
                total = len(fillers)
                state = {'emitted': 0, 'slot': 0}

                def drain():
                    tgt = total * (state['slot'] + 1) // slots
                    while state['emitted'] < tgt:
                        fillers[state['emitted']]()
                        state['emitted'] += 1
                    state['slot'] += 1

                for g in range(2):
                    atp = [at_pool.tile([P, SJ], f32, tag="at",
                                        name=f"at{j}{g}{h2}")
                           for h2 in range(2)]
                    prev = None
                    for cb in range(nblk):
                        drain()
                        col0 = max(0, cb - 4 * j) * P
                        # both heads' scores into one 2-bank PSUM tile so a
                        # single ACT instruction exps the pair
                        scp = sc_pool.tile([P, 2, SJ], f32, tag="sc",
                                           name=f"sc{j}{g}{cb}")
                        for h2 in range(2):
                            nc.tensor.matmul(
                                scp[:, h2, col0:],
                                KTH[g][h2][:, bass.ts(cb, P)],
                                QT[g][:, j * SJ + col0:(j + 1) * SJ],
                                start=True, stop=True)
                        ex = ex_pool.tile([P, 2, SJ], V_DT, tag="ex",
                                          name=f"ex{j}{g}{cb}")
                        nc.scalar.activation(
                            ex[:, :, col0:], scp[:, :, col0:], EXP,
                            scale=1.0 / (32.0 * W_SCALE * W_SCALE))
                        if cb >= 4 * j:
                            # causal mask on the 128-wide diagonal strip:
                            # keep where s_local - t_local >= 0 (gpsimd, so
                            # neither vector nor scalar queue is touched)
                            for h2 in range(2):
                                nc.gpsimd.affine_select(
                                    ex[:, h2, col0:col0 + P],
                                    ex[:, h2, col0:col0 + P],
                                    pattern=[[1, P]],
                                    compare_op=mybir.AluOpType.is_ge,
                                    fill=0.0, base=0, channel_multiplier=-1)
                        if prev is not None:
                            pcb, pcol0, pex = prev
                            for h2 in range(2):
                                nc.tensor.matmul(
                                    atp[h2][:, pcol0:],
                                    Vall[:, pcb, h2, g, :], pex[:, h2, pcol0:],
                                    start=(pcb == 0), stop=False)
                        prev = (cb, col0, ex)
                    pcb, pcol0, pex = prev
                    for h2 in range(2):
                        nc.tensor.matmul(
                            atp[h2][:, pcol0:],
                            Vall[:, pcb, h2, g, :], pex[:, h2, pcol0:],
                            start=(pcb == 0), stop=True)
                    epilogue(j, g, atp)
                # leftovers
                while state['emitted'] < total:
                    fillers[state['emitted']]()
                    state['emitted'] += 1

            # ---------------- schedule ----------------
            # tile 0 projections run un-hidden (nothing to hide behind)
            for f in proj_fillers(0, init=True):
                f()
            attention(0, proj_fillers(1))
            attention(1, proj_fillers(2))
            attention(2, proj_fillers(3) + outproj_fillers(0))
            attention(3, outproj_fillers(1) + outproj_fillers(2))
            for f in outproj_fillers(3):
                f()

    nc.compile()
    return nc


def _get_nc():
    if not _NC_CACHE:
        _NC_CACHE.append(_build())
    return _NC_CACHE[0]


def _tile_x(xb, np_dt):
    # [D, S] -> [NJ, P, ND, SJ]: arr[j, p, o, s] = xb[o*P+p, j*SJ+s]
    return np.ascontiguousarray(
        xb.reshape(ND, P, NJ, SJ).transpose(2, 1, 0, 3)).astype(np_dt)


def _tile_w(w, np_dt):
    # [D, M] -> [P, ND*M]: row p = concat_o w[o*P+p, :]
    m = w.shape[1]
    return np.ascontiguousarray(
        w.reshape(ND, P, m).transpose(1, 0, 2).reshape(P, ND * m)).astype(np_dt)


def _in_maps(q, k, v, W_q, W_k, W_v, W_o):
    x_np = _NP_OF[X_DT]
    v_np = _NP_OF[V_DT]
    tri = (np.arange(P)[:, None] <= np.arange(P)[None, :]).astype(v_np)
    xT = {}
    for b in range(B):
        xT[b] = (
            _tile_x(q[b].T, x_np),
            _tile_x(k[b].T, x_np),
            _tile_x(v[b].T, v_np),
        )
    maps = []
    for core in range(NCORES):
        b, quad = divmod(core, 4)
        hs = slice(4 * quad, 4 * quad + 4)
        qT_b, kT_b, vT_b = xT[b]
        # [4, D, E] -> [D, 4, E] -> [D, 256], col l*64+e = W[4q+l, d, e]
        wq = W_q[hs].transpose(1, 0, 2).reshape(D, 4 * E) * W_SCALE
        wk = W_k[hs].transpose(1, 0, 2).reshape(D, 4 * E) * W_SCALE
        wv = W_v[hs].transpose(1, 0, 2).reshape(D, 4 * E)
        # W_o[out, in] -> W_o.T rows for this quad's 256 input dims,
        # pre-tiled to [P, 2*D]: row p = [wotT[p, :], wotT[P+p, :]]
        wot = W_o[:, 4 * quad * E:4 * quad * E + 4 * E].T
        wot = wot.reshape(2, P, D).transpose(1, 0, 2).reshape(P, 2 * D)
        maps.append({
            "qT": qT_b,
            "kT": kT_b,
            "vT": vT_b,
            "wq": _tile_w(wq, x_np),
            "wk": _tile_w(wk, x_np),
            "wv": _tile_w(wv, v_np),
            "wot": np.ascontiguousarray(wot).astype(v_np),
            "tri": tri,
        })
    return maps


def kernel(q, k, v, W_q, W_k, W_v, W_o, _trace=False, _trace_kwargs=None):
    q = np.asarray(q, dtype=np.float32)
    k = np.asarray(k, dtype=np.float32)
    v = np.asarray(v, dtype=np.float32)
    W_q = np.asarray(W_q, dtype=np.float32)
    W_k = np.asarray(W_k, dtype=np.float32)
    W_v = np.asarray(W_v, dtype=np.float32)
    W_o = np.asarray(W_o, dtype=np.float32)

    nc = _get_nc()
    maps = _in_maps(q, k, v, W_q, W_k, W_v, W_o)
    kwargs = dict(_trace_kwargs or {})
    res = run_bass_kernel_spmd(
        nc, maps, core_ids=list(range(NCORES)), trace=_trace, **kwargs)
    out = np.zeros((B, S, D), dtype=np.float32)
    for core in range(NCORES):
        b = core // 4
        out[b] += res.results[core]["out"].astype(np.float32)
    if _trace:
        kernel.last_results = res
    return out


# revision 39
# speedup vs baseline: 1.0704x; 1.0704x over previous
"""Causal multi-head attention (nn_Attention_87840671138123) on 8 trn2 NeuronCores.

Problem (B=2, S=2048, D=1024, H=16 heads, E=64 head_dim), fp32:
    Q = einsum('bsd,hde->bhse', q, W_q)   (same for K, V)
    scores = Q @ K^T / sqrt(D), causal mask, softmax
    attn = probs @ V  -> [B, S, D] (head-major concat)
    out = attn @ W_o.T

Sharding: core = 4*b + quad. Each core handles batch b and a quad of 4 heads,
computes out_part = attn_quad @ W_o.T[quad rows, :], and the host sums the 4
partials per batch (the output-projection all-reduce done at gather time).

v2 schedule (vs the 233us baseline): the attention inner loop is
ACT(exp)-throughput-bound, so all projection matmuls for s-tile j+1 and the
output-projection matmuls for earlier tiles are emitted as *fillers*
interleaved into tile j's score/attn matmul stream; the attnV pair for block
cb is emitted one block behind its scores (skew) so exp latency is hidden.
Epilogue: reciprocal_approx_fast on the PSUM denominator row (lane-locked),
one gpsimd partition_broadcast, one multiply straight into attnG - no
copy+DMA+6-pass reciprocal chain.  V_aug for head pair h2=1 keeps its values
in PSUM partitions 64..127 (ones col at 63) so both normalize multiplies are
lane-aligned with attnG.  Output is written fp16 (cast is half the vector
time of an f32 copy, and half the HBM traffic); the host accumulates in f32.

Numerics: Q/K path bf16, V path fp16 (see baseline notes: fp32r trips the
chip power throttle).  exp folds the 1/sqrt(1024) scale.  fp16 out adds
~2e-4 relative error; harness gate is 2e-2.
"""

import ml_dtypes
import numpy as np

import concourse.bass as bass
import concourse.tile as tile
from concourse import bacc, mybir
from concourse.bass_utils import run_bass_kernel_spmd

B, S, D, H, E = 2, 2048, 1024, 16, 64
P = 128
NCORES = 8
SJ = 512            # s-tile width
NJ = S // SJ        # 4 s-tiles
ND = D // P         # 8 d-chunks
NT = S // P         # 16 t-chunks
f32 = mybir.dt.float32
bf16 = mybir.dt.bfloat16
fp16 = mybir.dt.float16
fp8 = mybir.dt.float8e4
EXP = mybir.ActivationFunctionType.Exp
MULT = mybir.AluOpType.mult
DROW = mybir.MatmulPerfMode.DoubleRow

X_DT = fp8          # q/k inputs + Wq/Wk: fp8 DoubleRow projections (2x PE).
W_SCALE = 8.0       # Wq/Wk pre-scaled by 8 to clear the fp8 subnormal floor;
                    # folded back via the exp scale (scores carry 64x).
QK_DT = bf16        # QT/KT activations, scores matmul
V_DT = fp16         # v input, Wv, V_aug, expT, attnG, WoT
OUT_DT = fp16       # device output; host accumulates in f32

_NP_OF = {bf16: ml_dtypes.bfloat16, fp16: np.float16, f32: np.float32,
          fp8: ml_dtypes.float8_e4m3}

_NC_CACHE = []


def _patch_ldw_opt():
    """Enable walrus LDWEIGHTS optimization (fast weight load).

    bass_utils.bir_verify_and_optimise hardcodes --enable-ldw-opt=false;
    wrap it to rewrite the flag. Verified numerically by the test harness.
    """
    from concourse import bass_utils as _bu
    if getattr(_bu, "_ldw_patched", False):
        return
    _orig_run = _bu.run_command

    def _run(argv, **kw):
        argv = ["--enable-ldw-opt=true" if a == "--enable-ldw-opt=false" else a
                for a in argv]
        return _orig_run(argv, **kw)

    _bu.run_command = _run
    _bu._ldw_patched = True


def _build():
    nc = bacc.Bacc("TRN2", target_bir_lowering=False, debug=False)

    # all inputs are host-pre-tiled to the exact SBUF layout so every DMA
    # moves long contiguous rows (512B rows run at ~19GB/s vs ~350GB/s for
    # 4-8KB rows)
    qT_d = nc.dram_tensor("qT", [NJ, P, ND, SJ], X_DT, kind="ExternalInput")
    kT_d = nc.dram_tensor("kT", [NJ, P, ND, SJ], X_DT, kind="ExternalInput")
    vT_d = nc.dram_tensor("vT", [NJ, P, ND, SJ], V_DT, kind="ExternalInput")
    wq_d = nc.dram_tensor("wq", [P, ND * 4 * E], X_DT, kind="ExternalInput")
    wk_d = nc.dram_tensor("wk", [P, ND * 4 * E], X_DT, kind="ExternalInput")
    wv_d = nc.dram_tensor("wv", [P, ND * 4 * E], V_DT, kind="ExternalInput")
    wot_d = nc.dram_tensor("wot", [P, 2 * D], V_DT, kind="ExternalInput")
    tri_d = nc.dram_tensor("tri", [P, P], V_DT, kind="ExternalInput")
    out_d = nc.dram_tensor("out", [S, D], OUT_DT, kind="ExternalOutput")

    with tile.TileContext(nc) as tc:
        with (
            tc.tile_pool(name="pers", bufs=1) as pers,
            tc.tile_pool(name="xt", bufs=2) as xt_pool,
            tc.tile_pool(name="ex", bufs=6) as ex_pool,
            tc.tile_pool(name="sm", bufs=4) as sm_pool,
            tc.tile_pool(name="ot", bufs=3) as ot_pool,
            tc.tile_pool(name="pj", bufs=2, space="PSUM") as pj_pool,
            tc.tile_pool(name="sc", bufs=2, space="PSUM") as sc_pool,
            tc.tile_pool(name="at", bufs=4, space="PSUM") as at_pool,
        ):
            # ---- persistent weights / constants ----
            wq_sb = pers.tile([P, ND, 4 * E], X_DT, name="wq_sb")
            wk_sb = pers.tile([P, ND, 4 * E], X_DT, name="wk_sb")
            wv_sb = pers.tile([P, ND, 4 * E], V_DT, name="wv_sb")
            wot_sb = pers.tile([P, 2, D], V_DT, name="wot_sb")
            tri_sb = pers.tile([P, P], V_DT, name="tri_sb")

            # ---- persistent activations ----
            QT = [pers.tile([P, S], QK_DT, name=f"QT{g}") for g in range(2)]
            # Per-head KT zero-padded to 128 partitions (rows of the other
            # head hold ~1e-20) so the scores matmul runs K=128: full PE rows
            # keep the HAM activity monitor from dropping the clock to 1.2GHz.
            KTH = [[pers.tile([P, S], QK_DT, name=f"KT{g}{h2}") for h2 in range(2)]
                   for g in range(2)]
            # V_aug blocks indexed [t, h2, g]: V in cols 0..63, ones at col 64
            # (denominator -> PSUM partition 64).  All pad cols hold 1.0 (PE
            # activity + they only feed unused PSUM partitions).  NB hardware
            # gpsimd/custom-DVE ucode does not honor partition offsets on
            # some APs, so the epilogue only uses baseline-proven shapes.
            Vall = pers.tile([P, NT, 2, 2, P], V_DT, name="Vall")
            attnG = [pers.tile([P, S], V_DT, name=f"attnG{g}") for g in range(2)]

            # weight DMAs ride the Activation HWDGE queue so they overlap the
            # x-input DMAs on the SP/Sync queue during startup
            nc.scalar.dma_start(wq_sb[:], wq_d.ap())
            nc.scalar.dma_start(wk_sb[:], wk_d.ap())
            nc.scalar.dma_start(wv_sb[:], wv_d.ap())
            nc.scalar.dma_start(tri_sb[:], tri_d.ap())
            nc.scalar.dma_start(wot_sb[:], wot_d.ap())

            # ---------------- helpers ----------------
            def start_x_dma(j):
                xq = xt_pool.tile([P, ND, SJ], X_DT, tag="xq", name=f"xq{j}")
                nc.sync.dma_start(xq[:], qT_d.ap()[j])
                xk = xt_pool.tile([P, ND, SJ], X_DT, tag="xk", name=f"xk{j}")
                nc.sync.dma_start(xk[:], kT_d.ap()[j])
                xv = xt_pool.tile([P, ND, SJ], V_DT, tag="xv", name=f"xv{j}")
                nc.sync.dma_start(xv[:], vT_d.ap()[j])
                return xq, xk, xv

            # Q/K projections run fp8 DoubleRow: each matmul contracts two
            # d-chunks (K=256) at 2 elem/partition/cycle -> half the PE time
            def q_step(c, g, j, js, xq, cell):
                if c == 0:
                    cell['t'] = pj_pool.tile([P, SJ], f32, tag="pj",
                                             name=f"pq{j}{g}")
                nc.tensor.matmul(cell['t'][:],
                                 wq_sb[:, 2 * c:2 * c + 2, bass.ts(g, P)],
                                 xq[:, 2 * c:2 * c + 2, :],
                                 start=(c == 0), stop=(c == ND // 2 - 1),
                                 perf_mode=DROW)
                if c == ND // 2 - 1:
                    nc.vector.tensor_copy(QT[g][:, js], cell['t'][:])

            def k_step(c, g, j, js, xk, cell):
                if c == 0:
                    cell['t'] = pj_pool.tile([P, SJ], f32, tag="pj",
                                             name=f"pk{j}{g}")
                nc.tensor.matmul(cell['t'][:],
                                 wk_sb[:, 2 * c:2 * c + 2, bass.ts(g, P)],
                                 xk[:, 2 * c:2 * c + 2, :],
                                 start=(c == 0), stop=(c == ND // 2 - 1),
                                 perf_mode=DROW)
                if c == ND // 2 - 1:
                    nc.vector.tensor_copy(KTH[g][0][0:E, js], cell['t'][0:E, :])
                    nc.vector.tensor_copy(KTH[g][1][E:2 * E, js],
                                          cell['t'][E:2 * E, :])

            def v_step(c, u, j, xv, cell):
                if c == 0:
                    cell['t'] = pj_pool.tile([P, 2, 2, E], f32, tag="pj",
                                             name=f"pv{j}{u}")
                nc.tensor.matmul(cell['t'][:], xv[:, c, bass.ts(u, P)],
                                 wv_sb[:, c, :], start=(c == 0), stop=(c == ND - 1))
                if c == ND - 1:
                    t = 4 * j + u
                    pv = cell['t']
                    # one cast for all four (h2, g) blocks: V into cols 0..63
                    nc.vector.tensor_copy(
                        Vall[:, t, :, :, 0:E],
                        pv[:].rearrange("p g h e -> p h g e"))

            def proj_fillers(j, init=False):
                x = start_x_dma(j)
                xq, xk, xv = x
                js = slice(j * SJ, (j + 1) * SJ)
                fl = []
                for g in range(2):
                    cell = {}
                    for c in range(ND // 2):
                        fl.append(lambda c=c, g=g, cell=cell:
                                  q_step(c, g, j, js, xq, cell))
                if init:
                    # one-time pads, placed here so they don't delay the
                    # first QT casts on the vector queue
                    for g in range(2):
                        fl.append(lambda g=g: nc.vector.memset(
                            KTH[g][0][E:2 * E, :], 1e-20))
                        fl.append(lambda g=g: nc.vector.memset(
                            KTH[g][1][0:E, :], 1e-20))
                for g in range(2):
                    cell = {}
                    for c in range(ND // 2):
                        fl.append(lambda c=c, g=g, cell=cell:
                                  k_step(c, g, j, js, xk, cell))
                if init:
                    # V_aug pad cols (ones col E + 1.0 filler above it); the
                    # V value cols are fully overwritten by the casts
                    fl.append(lambda: nc.vector.memset(
                        Vall[:, :, :, :, E:P], 1.0))
                for u in range(SJ // P):
                    cell = {}
                    for c in range(ND):
                        fl.append(lambda c=c, u=u, cell=cell:
                                  v_step(c, u, j, xv, cell))
                return fl

            def o_step(si, no, k, cell):
                if k == 0:
                    cell['t'] = pj_pool.tile([P, SJ], f32, tag="pj",
                                             name=f"po{si}{no}")
                    if no == 0:
                        cell['ot'] = ot_pool.tile([P, D], OUT_DT, tag="ot",
                                                  name=f"ot{si}")
                nc.tensor.matmul(cell['t'][:], attnG[k][:, bass.ts(si, P)],
                                 wot_sb[:, k, bass.ts(no, SJ)],
                                 start=(k == 0), stop=(k == 1))
                if k == 1:
                    nc.vector.tensor_copy(cell['ot'][:, bass.ts(no, SJ)],
                                          cell['t'][:])
                    if no == 1:
                        # one 2KB-row DMA per 128-row output stripe
                        nc.sync.dma_start(out_d.ap()[bass.ts(si, P), :],
                                          cell['ot'][:])

            def outproj_fillers(jp):
                fl = []
                for u in range(SJ // P):
                    si = 4 * jp + u
                    cell = {}  # shared per si: 'ot' spans both no-chunks
                    for no in range(2):
                        for k in range(2):
                            fl.append(lambda si=si, no=no, k=k, cell=cell:
                                      o_step(si, no, k, cell))
                return fl

            def epilogue(j, g, atp):
                js = slice(j * SJ, (j + 1) * SJ)
                for h2 in (1, 0):
                    den = sm_pool.tile([E + 1, SJ], f32, tag="den",
                                       name=f"den{j}{g}{h2}")
                    rec0 = sm_pool.tile([1, SJ], f32, tag="rec0",
                                        name=f"rec0{j}{g}{h2}")
                    nc.vector.tensor_copy(den[E:E + 1, :], atp[h2][E:E + 1, :])
                    nc.sync.dma_start(rec0[:], den[E:E + 1, :])
                    # 1/den: 18-bit accurate, ~5x faster than the 6-pass
                    # reciprocal (den >= 1, far from the undefined edge cases)
                    nc.vector.reciprocal_approx_fast(rec0[:], rec0[:])
                    recb = sm_pool.tile([E, SJ], f32, tag="recb",
                                        name=f"recb{j}{g}{h2}")
                    nc.gpsimd.partition_broadcast(recb[:], rec0[:])
                    if h2 == 0:
                        nc.vector.tensor_tensor(
                            attnG[g][0:E, js], atp[h2][0:E, :], recb[:], MULT)
                    else:
                        ah = sm_pool.tile([E, SJ], V_DT, tag="ah",
                                          name=f"ah{j}{g}")
                        nc.vector.tensor_tensor(
                            ah[:], atp[h2][0:E, :], recb[:], MULT)
                        nc.sync.dma_start(attnG[g][E:2 * E, js], ah[:])

            def attention(j, fillers):
                nblk = 4 * j + 4
                slots = 2 * nblk
                total = len(fillers)
                state = {'emitted': 0, 'slot': 0}

                def drain():
                    tgt = total * (state['slot'] + 1) // slots
                    while state['emitted'] < tgt:
                        fillers[state['emitted']]()
                        state['emitted'] += 1
                    state['slot'] += 1

                for g in range(2):
                    atp = [at_pool.tile([P, SJ], f32, tag="at",
                                        name=f"at{j}{g}{h2}")
                           for h2 in range(2)]
                    prev = None
                    for cb in range(nblk):
                        drain()
                        col0 = max(0, cb - 4 * j) * P
                        scps = []
                        for h2 in range(2):
                            scp = sc_pool.tile([P, SJ], f32, tag="sc",
                                               name=f"sc{j}{g}{cb}{h2}")
                            nc.tensor.matmul(
                                scp[:, col0:],
                                KTH[g][h2][:, bass.ts(cb, P)],
                                QT[g][:, j * SJ + col0:(j + 1) * SJ],
                                start=True, stop=True)
                            scps.append(scp)
                        exs = []
                        for h2 in range(2):
                            ex = ex_pool.tile([P, SJ], V_DT, tag="ex",
                                              name=f"ex{j}{g}{cb}{h2}")
                            nc.scalar.activation(
                                ex[:, col0:], scps[h2][:, col0:], EXP,
                                scale=1.0 / (32.0 * W_SCALE * W_SCALE))
                            if cb >= 4 * j:
                                nc.vector.tensor_tensor(
                                    ex[:, col0:col0 + P], ex[:, col0:col0 + P],
                                    tri_sb[:], MULT)
                            exs.append(ex)
                        if prev is not None:
                            pcb, pcol0, pexs = prev
                            for h2 in range(2):
                                nc.tensor.matmul(
                                    atp[h2][:, pcol0:],
                                    Vall[:, pcb, h2, g, :], pexs[h2][:, pcol0:],
                                    start=(pcb == 0), stop=False)
                        prev = (cb, col0, exs)
                    pcb, pcol0, pexs = prev
                    for h2 in range(2):
                        nc.tensor.matmul(
                            atp[h2][:, pcol0:],
                            Vall[:, pcb, h2, g, :], pexs[h2][:, pcol0:],
                            start=(pcb == 0), stop=True)
                    epilogue(j, g, atp)
                # leftovers
                while state['emitted'] < total:
                    fillers[state['emitted']]()
                    state['emitted'] += 1

            # ---------------- schedule ----------------
            # tile 0 projections run un-hidden (nothing to hide behind)
            for f in proj_fillers(0, init=True):
                f()
            attention(0, proj_fillers(1))
            attention(1, proj_fillers(2))
            attention(2, proj_fillers(3) + outproj_fillers(0))
            attention(3, outproj_fillers(1) + outproj_fillers(2))
            for f in outproj_fillers(3):
                f()

    nc.compile()
    return nc


def _get_nc():
    if not _NC_CACHE:
        _NC_CACHE.append(_build())
    return _NC_CACHE[0]


def _tile_x(xb, np_dt):
    # [D, S] -> [NJ, P, ND, SJ]: arr[j, p, o, s] = xb[o*P+p, j*SJ+s]
    return np.ascontiguousarray(
        xb.reshape(ND, P, NJ, SJ).transpose(2, 1, 0, 3)).astype(np_dt)


def _tile_w(w, np_dt):
    # [D, M] -> [P, ND*M]: row p = concat_o w[o*P+p, :]
    m = w.shape[1]
    return np.ascontiguousarray(
        w.reshape(ND, P, m).transpose(1, 0, 2).reshape(P, ND * m)).astype(np_dt)


def _in_maps(q, k, v, W_q, W_k, W_v, W_o):
    x_np = _NP_OF[X_DT]
    v_np = _NP_OF[V_DT]
    tri = (np.arange(P)[:, None] <= np.arange(P)[None, :]).astype(v_np)
    xT = {}
    for b in range(B):
        xT[b] = (
            _tile_x(q[b].T, x_np),
            _tile_x(k[b].T, x_np),
            _tile_x(v[b].T, v_np),
        )
    maps = []
    for core in range(NCORES):
        b, quad = divmod(core, 4)
        hs = slice(4 * quad, 4 * quad + 4)
        qT_b, kT_b, vT_b = xT[b]
        # [4, D, E] -> [D, 4, E] -> [D, 256], col l*64+e = W[4q+l, d, e]
        wq = W_q[hs].transpose(1, 0, 2).reshape(D, 4 * E) * W_SCALE
        wk = W_k[hs].transpose(1, 0, 2).reshape(D, 4 * E) * W_SCALE
        wv = W_v[hs].transpose(1, 0, 2).reshape(D, 4 * E)
        # W_o[out, in] -> W_o.T rows for this quad's 256 input dims,
        # pre-tiled to [P, 2*D]: row p = [wotT[p, :], wotT[P+p, :]]
        wot = W_o[:, 4 * quad * E:4 * quad * E + 4 * E].T
        wot = wot.reshape(2, P, D).transpose(1, 0, 2).reshape(P, 2 * D)
        maps.append({
            "qT": qT_b,
            "kT": kT_b,
            "vT": vT_b,
            "wq": _tile_w(wq, x_np),
            "wk": _tile_w(wk, x_np),
            "wv": _tile_w(wv, v_np),
            "wot": np.ascontiguousarray(wot).astype(v_np),
            "tri": tri,
        })
    return maps


def kernel(q, k, v, W_q, W_k, W_v, W_o, _trace=False, _trace_kwargs=None):
    q = np.asarray(q, dtype=np.float32)
    k = np.asarray(k, dtype=np.float32)
    v = np.asarray(v, dtype=np.float32)
    W_q = np.asarray(W_q, dtype=np.float32)
    W_k = np.asarray(W_k, dtype=np.float32)
    W_v = np.asarray(W_v, dtype=np.float32)
    W_o = np.asarray(W_o, dtype=np.float32)

    nc = _get_nc()
    maps = _in_maps(q, k, v, W_q, W_k, W_v, W_o)
    kwargs = dict(_trace_kwargs or {})
    res = run_bass_kernel_spmd(
        nc, maps, core_ids=list(range(NCORES)), trace=_trace, **kwargs)
    out = np.zeros((B, S, D), dtype=np.float32)
    for core in range(NCORES):
        b = core // 4
        out[b] += res.results[core]["out"].astype(np.float32)
    if _trace:
        kernel.last_results = res
    return out
